# revision 15
# baseline (speedup 1.0000x reference)
"""Trainium2 Bass kernel for nn_DiffusionFlowEmbedder.

Computes: KLDivLoss(Pg^4 || Pe^4)/N + mean((decoder(encoder(X)) - X)^2)  (scalar)

Distribution (8 NeuronCores, SPMD, full inputs replicated + a per-core row
shard of X):
  - Each core owns 256 rows (the n2 axis) of both pairwise-affinity matrices
    in transposed convention: row-shards of Q = Pg^T and B = Pe^T.
  - Pairwise lengths/dot products come from augmented Gram matmuls on TensorE
    (||xj-xi||^2 = -2 xj.xi + sq_i + sq_j fused into a single PSUM pass).
  - Matrix 4th power = two squarings: AllGather bf16 shards, PE-transpose own
    shard for the stationary side, [256,2048]@[2048,2048] bf16 matmuls with
    fp32 PSUM accumulation.

Pipeline (v4), arranged so the serial B-side chain
  XTa -> encoder -> flow MLP -> stats -> Pe affinity -> AG(Bsh raw)
  -> B^2 matmuls -> AG(B^2) -> B^4 matmuls (KLD fused)
is as short as possible and everything else (Q affinity, AG(Qsh), Q^2/Q^4
matmuls, AG(Q2), decoder/recon) fills engine gaps underneath it:
  - flows are NOT pre-normalized; 1/||fl|| is folded into the affinity as a
    free-dim scale (replicated row), removing two serial stages.
  - The Pe matrix is gathered RAW; row-normalization diag(1/rowsum) is
    applied inside the B^2 squaring: gathered ktiles are scaled by r along
    partitions, PSUM output chunks by the replicated r row along free dim.
    The stationary transposes therefore depend only on the raw affinity and
    overlap the gather; the colsum AllReduce only gates the cheap ktile
    scale.
  - KLD is fused into the B^4 squaring, reading the materialized Q^4.
  - Per-core KLD partials are summed on the host (no final AllReduce).

CC queue: AG(Qsh) -> AR(colsum) -> AG(Bsh) -> AG(Q2) -> AG(B2).

Hardware constraint honored throughout: compute-engine SBUF access patterns
must start at partition 0/32/64/96, so scalar rows that land on other
partitions (augmented-matmul extra rows) are staged in a base-0 scratch row
and moved with SBUF-to-SBUF DMA (DMA has no base-partition restriction).
"""
import sys
import functools

sys.path.insert(0, "/opt/trn_rl_repo")

import numpy as np

import concourse.bass as bass
import concourse.bacc as bacc
import concourse.mybir as mybir
import concourse.tile as tile
import concourse.masks as masks
from concourse.bass_utils import run_bass_kernel_spmd

F32 = mybir.dt.float32
B16 = mybir.dt.bfloat16
AF = mybir.ActivationFunctionType
OP = mybir.AluOpType
AX = mybir.AxisListType

N, D, EMB = 2048, 100, 2
NCORES = 8
S = N // NCORES           # 256 rows per core
P = 128
NB = N // P               # 16 partition blocks of the full matrix
SB = S // P               # 2 partition blocks of a shard
CH = 512                  # free-dim chunk
NCH = N // CH             # 4 chunks
EPS = 1e-12
SIG = 0.5
FS_G = 5.0
PG_CLAMP = 1e-4           # len^2 floor (Pg side; X scale ~10)
PE_CLAMP = 1e-5           # len^2 floor (Pe side; emb scale ~0.3)
AE = [100, 10]
FA = [10, 20, 10]

WSPECS = [
    ("eW0", [D, AE[0]]), ("eb0", [AE[0], 1]),
    ("eW1", [AE[0], AE[1]]), ("eb1", [AE[1], 1]),
    ("eW2", [AE[1], EMB]), ("eb2", [EMB, 1]),
    ("dW0", [EMB, AE[1]]), ("db0", [AE[1], 1]),
    ("dW1", [AE[1], AE[0]]), ("db1", [AE[0], 1]),
    ("dW2", [AE[0], D]), ("db2", [D, 1]),
    ("fW0", [EMB, FA[0]]), ("fb0", [FA[0], 1]),
    ("fW1", [FA[0], FA[1]]), ("fb1", [FA[1], 1]),
    ("fW2", [FA[1], FA[2]]), ("fb2", [FA[2], 1]),
    ("fW3", [FA[2], EMB]), ("fb3", [EMB, 1]),
]

# stat-tile partitions (rows used as matmul rhs must start at 0/32/64/96)
SA_FSQ, SA_FSQE, SA_RA, SA_CSUM = 0, 32, 64, 96


def _build(fs_value: float, debug_names=()):
    nc = bacc.Bacc(
        "TRN2", target_bir_lowering=False, debug=False,
        enable_asserts=False, num_devices=NCORES,
    )
    dX = nc.dram_tensor("X", [N, D], F32, kind="ExternalInput")
    dF = nc.dram_tensor("flows", [N, D], F32, kind="ExternalInput")
    dXs = nc.dram_tensor("Xshard", [S, D], F32, kind="ExternalInput")
    dOnes = nc.dram_tensor("ones2048", [1, N], F32, kind="ExternalInput")
    dW = {nm: nc.dram_tensor(nm, sh, F32, kind="ExternalInput") for nm, sh in WSPECS}
    dOut = nc.dram_tensor("out", [1, 2], F32, kind="ExternalOutput")

    rg = [list(range(NCORES))]

    with tile.TileContext(nc) as tc:
        with (
            tc.tile_pool(name="main", bufs=1) as mp,
            tc.tile_pool(name="stream", bufs=2) as sp,
            tc.tile_pool(name="dram", bufs=1, space="DRAM") as dp,
            tc.tile_pool(name="pt", bufs=2, space="PSUM") as ptp,
            tc.tile_pool(name="pmm", bufs=4, space="PSUM") as pmp,
            tc.tile_pool(name="psq", bufs=2, space="PSUM") as pqp,
        ):
            def pt(dt_=F32):
                return ptp.tile([P, P], dt_, tag="pt", name="pt_t")

            def pmm(p_, f_):
                return pmp.tile([p_, f_], F32, tag="pmm", name="pmm_t")

            def pone(f_):
                return pmp.tile([1, f_], F32, tag="pmm", name="pone_t")

            def dbg(name, ap, shape):
                if name in debug_names:
                    t = nc.dram_tensor("dbg_" + name, shape, ap.dtype,
                                       kind="ExternalOutput")
                    nc.sync.dma_start(t[:, :], ap)

            # ---------------- constants ----------------
            id_f = mp.tile([P, P], F32)
            id_b = mp.tile([P, P], B16)
            masks.make_identity(nc, id_f[:])
            masks.make_identity(nc, id_b[:])
            ones_col = mp.tile([P, 1], F32)
            nc.vector.memset(ones_col[:], 1.0)
            neg_ones_col = mp.tile([P, 1], F32)
            nc.vector.memset(neg_ones_col[:], -1.0)
            ones_colb = mp.tile([P, 1], B16)
            nc.vector.memset(ones_colb[:], 1.0)
            ones_row = mp.tile([65, P], F32)
            nc.vector.memset(ones_row[0:65, :], 1.0)
            negfs_g = mp.tile([P, 1], F32)
            nc.vector.memset(negfs_g[:], -FS_G)
            negfs_e = mp.tile([P, 1], F32)
            nc.vector.memset(negfs_e[:], -float(fs_value))

            statA = mp.tile([97, N], F32)

            # ---------------- weights (vector DMA queue) ----------
            w = {}
            for nm, sh in WSPECS:
                w[nm] = mp.tile(sh, F32, tag="w_" + nm, name="w_" + nm)
                nc.scalar.dma_start(w[nm][:], dW[nm][:, :])

            # ---------------- load X / flows, transpose ----------------
            # XTa rows 0..99 = X^T, row 100 = sqrow, row 101 = ones
            # FLTa rows 0..99 = raw fl^T, row 100 = -xffrow (raw)
            XTa = mp.tile([P, N], F32)
            FLTa = mp.tile([P, N], F32)
            nc.scalar.dma_start(XTa[D + 1:D + 2, :], dOnes[0:1, :])

            for b in range(NB):
                xr = sp.tile([P, D], F32, tag="xr", name="xr", bufs=3)
                nc.sync.dma_start(xr[:], dX[b * P:(b + 1) * P, :])
                ps = pt()
                nc.tensor.transpose(ps[0:D, :], xr[:], id_f[:])
                nc.scalar.copy(XTa[0:D, b * P:(b + 1) * P], ps[0:D, :])

            for b in range(NB):
                fr = sp.tile([P, D], F32, tag="fr", name="fr", bufs=3)
                nc.sync.dma_start(fr[:], dF[b * P:(b + 1) * P, :])
                ps = pt()
                nc.tensor.transpose(ps[0:D, :], fr[:], id_f[:])
                nc.scalar.copy(FLTa[0:D, b * P:(b + 1) * P], ps[0:D, :])

            def replicate_row(dst, stile, ridx, nch=NCH, cw=CH, np_=P):
                # dst [np_, nch*cw] <- broadcast of stile[ridx] over partitions
                for t in range(nch):
                    cs = slice(t * cw, (t + 1) * cw)
                    pr = pmm(np_, cw)
                    nc.tensor.matmul(pr[:], ones_row[ridx:ridx + 1, 0:np_],
                                     stile[ridx:ridx + 1, cs],
                                     start=True, stop=True)
                    nc.scalar.copy(dst[0:np_, cs], pr[:])

            # X sq row -> XTa[100]
            for t in range(NCH):
                cs = slice(t * CH, (t + 1) * CH)
                scr = sp.tile([D, CH], F32, tag="scr", name="scr", bufs=2)
                nc.vector.tensor_tensor(scr[:], XTa[0:D, cs], XTa[0:D, cs], OP.mult)
                po = pone(CH)
                nc.tensor.matmul(po[:], ones_col[0:D, :], scr[:], start=True, stop=True)
                sqc = sp.tile([1, CH], F32, tag="rowx", name="sqc", bufs=3)
                nc.scalar.copy(sqc[:], po[:])
                nc.scalar.dma_start(XTa[D:D + 1, cs], sqc[:])
            # flow norms: rfrep = 1/||fl_b|| broadcast [P, N]
            for t in range(NCH):
                cs = slice(t * CH, (t + 1) * CH)
                scr2 = sp.tile([D, CH], F32, tag="scr", name="scrf", bufs=2)
                nc.vector.tensor_tensor(scr2[:], FLTa[0:D, cs], FLTa[0:D, cs], OP.mult)
                po = pone(CH)
                nc.tensor.matmul(po[:], ones_col[0:D, :], scr2[:], start=True, stop=True)
                nc.scalar.copy(statA[SA_FSQ:SA_FSQ + 1, cs], po[:])
            nc.scalar.activation(statA[SA_FSQ:SA_FSQ + 1, :],
                                 statA[SA_FSQ:SA_FSQ + 1, :], AF.Sqrt)
            nc.vector.reciprocal_approx_fast(statA[SA_FSQ:SA_FSQ + 1, :],
                                             statA[SA_FSQ:SA_FSQ + 1, :])
            rfrep = mp.tile([P, N], F32, tag="rep", name="rfrep", bufs=1)
            replicate_row(rfrep, statA, SA_FSQ)
            # -xff row (raw): -x_b . flraw_b
            for t in range(NCH):
                cs = slice(t * CH, (t + 1) * CH)
                scr3 = sp.tile([D, CH], F32, tag="scr", name="scrx", bufs=2)
                nc.vector.tensor_tensor(scr3[:], XTa[0:D, cs], FLTa[0:D, cs], OP.mult)
                po = pone(CH)
                nc.tensor.matmul(po[:], neg_ones_col[0:D, :], scr3[:],
                                 start=True, stop=True)
                xffc = sp.tile([1, CH], F32, tag="rowx", name="xffc", bufs=3)
                nc.scalar.copy(xffc[:], po[:])
                nc.scalar.dma_start(FLTa[D:D + 1, cs], xffc[:])

            # ---------------- shard aug lhsT ----------------
            Aug1 = mp.tile([P, S], F32)     # 0..99=-2Xs^T, 100=ones, 101=sq_sh
            Aug2 = mp.tile([P, S], F32)     # 0..99=Xs^T, 100=ones
            nc.scalar.dma_start(Aug1[D:D + 1, :], dOnes[0:1, 0:S])
            nc.scalar.dma_start(Aug2[D:D + 1, :], dOnes[0:1, 0:S])
            for b in range(SB):
                xsr = sp.tile([P, D], F32, tag="xr", name="xsr", bufs=3)
                nc.sync.dma_start(xsr[:], dXs[b * P:(b + 1) * P, :])
                ps = pt()
                nc.tensor.transpose(ps[0:D, :], xsr[:], id_f[:])
                nc.scalar.mul(Aug1[0:D, b * P:(b + 1) * P], ps[0:D, :], -2.0)
                nc.scalar.copy(Aug2[0:D, b * P:(b + 1) * P], ps[0:D, :])
            scr4 = sp.tile([D, S], F32, tag="scrS", name="scr4", bufs=2)
            nc.vector.tensor_tensor(scr4[:], Aug2[0:D, :], Aug2[0:D, :], OP.mult)
            po4 = pone(S)
            nc.tensor.matmul(po4[:], ones_col[0:D, :], scr4[:], start=True, stop=True)
            sshc = sp.tile([1, S], F32, tag="rowx", name="sshc", bufs=3)
            nc.scalar.copy(sshc[:], po4[:])
            nc.scalar.dma_start(Aug1[D + 1:D + 2, :], sshc[:])

            # ---------------- encoder MLP ----------------
            def dense(rhs_ap, nm_w, nm_b, fo, act, out_tag, width=N, out=None):
                if out is None:
                    out = mp.tile([fo, width], F32, tag=out_tag, name=out_tag,
                                  bufs=1)
                nch = max(width // CH, 1)
                cw = width // nch
                for t in range(nch):
                    cs = slice(t * cw, (t + 1) * cw)
                    pm = pmm(fo, cw)
                    nc.tensor.matmul(pm[0:fo, 0:cw], w[nm_w][:, :], rhs_ap[:, cs],
                                     start=True, stop=True)
                    nc.scalar.activation(out[0:fo, cs], pm[0:fo, 0:cw], act,
                                         bias=w[nm_b][:, 0:1], scale=1.0)
                return out

            H1T = dense(XTa[0:D, :], "eW0", "eb0", AE[0], AF.Relu, "mlpA")
            H2T = dense(H1T[:, :], "eW1", "eb1", AE[1], AF.Relu, "mlpB")
            # embTa rows 0..1 = embT, 2 = sqerow, 3 = ones
            embTa = mp.tile([4, N], F32)
            nc.scalar.dma_start(embTa[EMB + 1:EMB + 2, :], dOnes[0:1, :])
            for t in range(NCH):
                cs = slice(t * CH, (t + 1) * CH)
                pm = pmm(EMB, CH)
                nc.tensor.matmul(pm[0:EMB, :], w["eW2"][:, :], H2T[:, cs],
                                 start=True, stop=True)
                nc.scalar.activation(embTa[0:EMB, cs], pm[0:EMB, :], AF.Identity,
                                     bias=w["eb2"][:, 0:1], scale=1.0)
            # emb sq row -> embTa[2]
            for t in range(NCH):
                cs = slice(t * CH, (t + 1) * CH)
                scrE = sp.tile([EMB, CH], F32, tag="scrE", name="scrE", bufs=2)
                nc.vector.tensor_tensor(scrE[:], embTa[0:EMB, cs], embTa[0:EMB, cs],
                                        OP.mult)
                po = pone(CH)
                nc.tensor.matmul(po[:], ones_col[0:EMB, :], scrE[:],
                                 start=True, stop=True)
                sqec = sp.tile([1, CH], F32, tag="rowx", name="sqec", bufs=3)
                nc.scalar.copy(sqec[:], po[:])
                nc.scalar.dma_start(embTa[EMB:EMB + 1, cs], sqec[:])

            # ---------------- affinity ----------------
            def affinity(dst, lhs1, k1, rhs1, lhs2, k2, rhs2, clamp, fs_,
                         negfs_col, fscale, stage_to=None, colsum_row=None):
                for m in range(SB):
                    ms = slice(m * P, (m + 1) * P)
                    for t in range(NCH):
                        cs = slice(t * CH, (t + 1) * CH)
                        p_len = pmm(P, CH)
                        nc.tensor.matmul(p_len[:], lhs1[0:k1, ms], rhs1[0:k1, cs],
                                         start=True, stop=True)
                        p_num = pmm(P, CH)
                        nc.tensor.matmul(p_num[:], lhs2[0:k2, ms], rhs2[0:k2, cs],
                                         start=True, stop=True)
                        ln2 = sp.tile([P, CH], F32, tag="ln2", name="ln2", bufs=3)
                        nc.vector.tensor_scalar_max(ln2[:], p_len[:], clamp)
                        lnt = sp.tile([P, CH], F32, tag="lnt", name="lnt", bufs=3)
                        nc.scalar.activation(lnt[:], ln2[:], AF.Sqrt)
                        nc.vector.reciprocal_approx_fast(ln2[:], lnt[:])
                        nc.vector.tensor_tensor(ln2[:], p_num[:], ln2[:], OP.mult)
                        # fold 1/||fl_b|| (raw flows in p_num)
                        nc.vector.tensor_tensor(ln2[:], ln2[:], fscale[:, cs],
                                                OP.mult)
                        # fs*|dot/len - 1| fused: |x*fs - fs|
                        nc.scalar.activation(ln2[:], ln2[:], AF.Abs,
                                             scale=float(fs_),
                                             bias=negfs_col[:, 0:1])
                        nc.vector.tensor_tensor(ln2[:], ln2[:], lnt[:], OP.add)
                        c0 = m * N + t * CH
                        nc.scalar.activation(dst[:, c0:c0 + CH], ln2[:], AF.Exp,
                                             scale=-1.0 / SIG)
                        if stage_to is not None:
                            nc.sync.dma_start(
                                stage_to[m * P:(m + 1) * P,
                                         t * CH:(t + 1) * CH],
                                dst[:, c0:c0 + CH])
                        if colsum_row is not None:
                            pc = pone(CH)
                            nc.tensor.matmul(pc[:], ones_colb[:, :],
                                             dst[:, c0:c0 + CH],
                                             start=True, stop=True)
                            cs_row = statA[colsum_row:colsum_row + 1,
                                           t * CH:(t + 1) * CH]
                            if m == 0:
                                nc.scalar.copy(cs_row, pc[:])
                            else:
                                nc.vector.tensor_tensor(cs_row, cs_row, pc[:],
                                                        OP.add)

            # ---------------- Q affinity + gather ----------------
            Qsh = mp.tile([P, SB * N], B16, tag="qsh", name="qsh", bufs=1)
            aginQ = dp.tile([S, N], B16, name="aginQ")
            agoQ = dp.tile([N, N], B16, name="agoQ", addr_space="Shared")
            affinity(Qsh, Aug1, D + 2, XTa, Aug2, D + 1, FLTa, PG_CLAMP, FS_G,
                     negfs_g, rfrep, stage_to=aginQ)
            dbg("Qsh", Qsh[:, :], [P, SB * N])
            nc.gpsimd.collective_compute(
                "AllGather", OP.bypass, replica_groups=rg,
                ins=[aginQ.opt()], outs=[agoQ.opt()])

            # ---------------- shard encoder + AugE ----------------
            H1sT = dense(Aug2[0:D, :], "eW0", "eb0", AE[0], AF.Relu, "mlpsA", width=S)
            H2sT = dense(H1sT[:, :], "eW1", "eb1", AE[1], AF.Relu, "mlpsB", width=S)
            embsT = mp.tile([EMB, S], F32)
            pm = pmm(EMB, S)
            nc.tensor.matmul(pm[0:EMB, 0:S], w["eW2"][:, :], H2sT[:, :],
                             start=True, stop=True)
            nc.scalar.activation(embsT[:, :], pm[0:EMB, 0:S], AF.Identity,
                                 bias=w["eb2"][:, 0:1], scale=1.0)
            AugE1 = mp.tile([4, S], F32)   # rows 0..1 = -2 embsT, 2=ones, 3=sqe_sh
            AugE2 = mp.tile([3, S], F32)   # rows 0..1 = embsT, 2 = ones
            nc.vector.tensor_scalar_mul(AugE1[0:EMB, :], embsT[:, :], -2.0)
            nc.scalar.dma_start(AugE1[EMB:EMB + 1, :], dOnes[0:1, 0:S])
            nc.vector.tensor_copy(AugE2[0:EMB, :], embsT[:, :])
            nc.scalar.dma_start(AugE2[EMB:EMB + 1, :], dOnes[0:1, 0:S])
            scrE4 = sp.tile([EMB, S], F32, tag="scrS", name="scrE4", bufs=2)
            nc.vector.tensor_tensor(scrE4[:], embsT[:, :], embsT[:, :], OP.mult)
            poE = pone(S)
            nc.tensor.matmul(poE[:], ones_col[0:EMB, :], scrE4[:],
                             start=True, stop=True)
            sqehc = sp.tile([1, S], F32, tag="rowx", name="sqehc", bufs=3)
            nc.scalar.copy(sqehc[:], poE[:])
            nc.scalar.dma_start(AugE1[EMB + 1:EMB + 2, :], sqehc[:])

            # ---------------- flow MLP ----------------
            F1T = dense(embTa[0:EMB, :], "fW0", "fb0", FA[0], AF.Tanh, "mlpA")
            F2T = dense(F1T[:, :], "fW1", "fb1", FA[1], AF.Tanh, "mlpB")
            F3T = dense(F2T[:, :], "fW2", "fb2", FA[2], AF.Tanh, "mlpA")
            # FETa rows 0..1 = raw flow_e^T, row 2 = -xffe (raw)
            FETa = mp.tile([3, N], F32)
            for t in range(NCH):
                cs = slice(t * CH, (t + 1) * CH)
                pm = pmm(EMB, CH)
                nc.tensor.matmul(pm[0:EMB, :], w["fW3"][:, :], F3T[:, cs],
                                 start=True, stop=True)
                nc.scalar.activation(FETa[0:EMB, cs], pm[0:EMB, :], AF.Identity,
                                     bias=w["fb3"][:, 0:1], scale=1.0)

            # flow_e norms -> rinvErep; raw -xffe row (parallel chains)
            for t in range(NCH):
                cs = slice(t * CH, (t + 1) * CH)
                scrF = sp.tile([EMB, CH], F32, tag="scrE", name="scrF", bufs=2)
                nc.vector.tensor_tensor(scrF[:], FETa[0:EMB, cs], FETa[0:EMB, cs],
                                        OP.mult)
                po = pone(CH)
                nc.tensor.matmul(po[:], ones_col[0:EMB, :], scrF[:],
                                 start=True, stop=True)
                nc.scalar.copy(statA[SA_FSQE:SA_FSQE + 1, cs], po[:])
            nc.scalar.activation(statA[SA_FSQE:SA_FSQE + 1, :],
                                 statA[SA_FSQE:SA_FSQE + 1, :], AF.Sqrt)
            nc.vector.tensor_scalar_max(statA[SA_FSQE:SA_FSQE + 1, :],
                                        statA[SA_FSQE:SA_FSQE + 1, :], EPS)
            nc.vector.reciprocal_approx_fast(statA[SA_FSQE:SA_FSQE + 1, :],
                                             statA[SA_FSQE:SA_FSQE + 1, :])
            rerep = mp.tile([P, N], F32, tag="rep", name="rerep", bufs=1)
            replicate_row(rerep, statA, SA_FSQE)
            for t in range(NCH):
                cs = slice(t * CH, (t + 1) * CH)
                scrX = sp.tile([EMB, CH], F32, tag="scrE", name="scrX", bufs=2)
                nc.vector.tensor_tensor(scrX[:], embTa[0:EMB, cs], FETa[0:EMB, cs],
                                        OP.mult)
                po = pone(CH)
                nc.tensor.matmul(po[:], neg_ones_col[0:EMB, :], scrX[:],
                                 start=True, stop=True)
                xfc = sp.tile([1, CH], F32, tag="rowx", name="xfc", bufs=3)
                nc.scalar.copy(xfc[:], po[:])
                nc.scalar.dma_start(FETa[EMB:EMB + 1, cs], xfc[:])

            # ---------------- Pe raw affinity, AR, gather ------
            Bsh = mp.tile([P, SB * N], B16, tag="bsh", name="bsh", bufs=1)
            aginB = dp.tile([S, N], B16, name="aginB")
            agoB = dp.tile([N, N], B16, name="agoB", addr_space="Shared")
            affinity(Bsh, AugE1, EMB + 2, embTa, AugE2, EMB + 1, FETa,
                     PE_CLAMP, fs_value, negfs_e, rerep,
                     stage_to=aginB, colsum_row=SA_CSUM)
            dbg("BshRaw", Bsh[:, :], [P, SB * N])

            ar_in = dp.tile([1, N], F32, name="ar_in")
            ar_out = dp.tile([1, N], F32, name="ar_out", addr_space="Shared")
            nc.scalar.dma_start(ar_in[:], statA[SA_CSUM:SA_CSUM + 1, :])
            nc.gpsimd.collective_compute(
                "AllReduce", OP.add, replica_groups=rg,
                ins=[ar_in.opt()], outs=[ar_out.opt()])

            nc.gpsimd.collective_compute(
                "AllGather", OP.bypass, replica_groups=rg,
                ins=[aginB.opt()], outs=[agoB.opt()])

            # ---------------- decoder + recon (fills gaps) ----------
            G1T = dense(embTa[0:EMB, :], "dW0", "db0", AE[1], AF.Relu, "mlpB")
            G2T = dense(G1T[:, :], "dW1", "db1", AE[0], AF.Relu, "mlpA")
            racc = mp.tile([D, NCH], F32)
            for t in range(NCH):
                cs = slice(t * CH, (t + 1) * CH)
                pm = pmm(D, CH)
                nc.tensor.matmul(pm[0:D, :], w["dW2"][:, :], G2T[:, cs],
                                 start=True, stop=True)
                xrt = sp.tile([D, CH], F32, tag="scr", name="xrt", bufs=2)
                nc.scalar.activation(xrt[:], pm[0:D, :], AF.Identity,
                                     bias=w["db2"][:, 0:1], scale=1.0)
                dif = sp.tile([D, CH], F32, tag="dif", name="dif", bufs=2)
                nc.vector.tensor_tensor(dif[:], xrt[:], XTa[0:D, cs], OP.subtract)
                dsq = sp.tile([D, CH], F32, tag="dif", name="dsq", bufs=2)
                nc.vector.tensor_tensor(dsq[:], dif[:], dif[:], OP.mult)
                nc.vector.reduce_sum(racc[:, t:t + 1], dsq[:], axis=AX.X, op=OP.add)
            rsum = mp.tile([D, 1], F32)
            nc.vector.reduce_sum(rsum[:], racc[:, :], axis=AX.X, op=OP.add)
            prec = pone(1)
            nc.tensor.matmul(prec[:], rsum[:, :], ones_col[0:D, 0:1],
                             start=True, stop=True)
            recon_sc = mp.tile([1, 1], F32)
            nc.scalar.mul(recon_sc[:], prec[:], 1.0 / (N * D))
            dbg("recon", recon_sc[:, :], [1, 1])

            # ---------------- square machinery ----------------
            def transpose_shard(agin):
                # dst tile (k*SB+m) = (shard block [m-rows, k-cols])^T, read
                # straight from the staged DRAM shard via the DMA crossbar
                dst = mp.tile([P, SB * N], B16, tag="msh", name="msh", bufs=2)
                for k in range(NB):
                    nc.scalar.dma_start_transpose(
                        dst[:, k * S:(k + 1) * S],
                        agin[0:S, k * P:(k + 1) * P])
                return dst

            def square(srcT, agout, out=None, kt_scale=None, out_scale=None,
                       consumer=None, stage_to=None):
                for t in range(NCH):
                    kt = []
                    for k in range(NB):
                        ktile = sp.tile([P, CH], B16, tag="agk", name="agk", bufs=24)
                        nc.sync.dma_start(
                            ktile[:], agout[k * P:(k + 1) * P,
                                            t * CH:(t + 1) * CH])
                        if kt_scale is not None:
                            nc.vector.tensor_scalar(ktile[:], ktile[:],
                                                    kt_scale[:, k:k + 1], None,
                                                    OP.mult)
                        kt.append(ktile)
                    for m in range(SB):
                        ps_ = pqp.tile([P, CH], F32, tag="psq", name="psq")
                        for k in range(NB):
                            nc.tensor.matmul(
                                ps_[:],
                                srcT[:, (k * SB + m) * P:(k * SB + m + 1) * P],
                                kt[k][:],
                                start=(k == 0), stop=(k == NB - 1))
                        c0 = m * N + t * CH
                        if consumer is not None:
                            consumer(m, t, ps_)
                        elif out_scale is not None:
                            nc.vector.tensor_tensor(
                                out[:, c0:c0 + CH], ps_[:],
                                out_scale[:, t * CH:(t + 1) * CH], OP.mult)
                        else:
                            nc.vector.tensor_copy(out[:, c0:c0 + CH], ps_[:])
                        if stage_to is not None and consumer is None:
                            nc.sync.dma_start(
                                stage_to[m * P:(m + 1) * P,
                                         t * CH:(t + 1) * CH],
                                out[:, c0:c0 + CH])

            # ---------------- Q2 square + gather ----------------
            QshT = transpose_shard(aginQ)
            Q2 = mp.tile([P, SB * N], B16, tag="q2", name="q2", bufs=1)
            aginQ2 = dp.tile([S, N], B16, name="aginQ2")
            agoQ2 = dp.tile([N, N], B16, name="agoQ2", addr_space="Shared")
            square(QshT, agoQ, out=Q2, stage_to=aginQ2)
            nc.gpsimd.collective_compute(
                "AllGather", OP.bypass, replica_groups=rg,
                ins=[aginQ2.opt()], outs=[agoQ2.opt()])

            # ---------------- r post-processing (after AR) ----------------
            nc.scalar.dma_start(statA[SA_RA:SA_RA + 1, :], ar_out[:])
            nc.vector.reciprocal_approx_fast(statA[SA_RA:SA_RA + 1, :],
                                             statA[SA_RA:SA_RA + 1, :])
            rArep = mp.tile([P, N], F32, tag="rep", name="rArep", bufs=1)
            replicate_row(rArep, statA, SA_RA)
            # rcol [P, NB]: rcol[p, k] = r[k*P + p] (ktile partition scale)
            r16 = mp.tile([16, P], F32)
            for k in range(NB):
                nc.scalar.dma_start(r16[k:k + 1, :],
                                      statA[SA_RA:SA_RA + 1, k * P:(k + 1) * P])
            psr = pt()
            nc.tensor.transpose(psr[:, 0:16], r16[0:16, :], id_f[0:16, 0:16])
            rcol = mp.tile([P, NB], F32)
            nc.scalar.copy(rcol[:, :], psr[:, 0:16])

            # ---------------- B2 square (normalization folded in) ----------
            BshT = transpose_shard(aginB)
            B2 = mp.tile([P, SB * N], B16, tag="b2", name="b2", bufs=1)
            aginB2 = dp.tile([S, N], B16, name="aginB2")
            agoB2 = dp.tile([N, N], B16, name="agoB2", addr_space="Shared")
            square(BshT, agoB, out=B2, kt_scale=rcol, out_scale=rArep,
                   stage_to=aginB2)
            dbg("B2", B2[:, :], [P, SB * N])
            nc.gpsimd.collective_compute(
                "AllGather", OP.bypass, replica_groups=rg,
                ins=[aginB2.opt()], outs=[agoB2.opt()])

            # ---------------- Q4 square (materialized) ----------------
            Q2T = transpose_shard(aginQ2)
            Q4 = mp.tile([P, SB * N], B16, tag="qsh", name="q4", bufs=1)
            square(Q2T, agoQ2, out=Q4)
            dbg("Q4", Q4[:, :], [P, SB * N])

            # ---------------- B4 square fused with KLD ----------------
            kacc = mp.tile([P, SB * NCH], F32)
            nc.vector.memset(kacc[:], 0.0)

            def kld_consumer(m, t, ps_):
                # ps_ = B4 [P, CH] chunk (fp32 PSUM); Q4 is materialized
                c0 = m * N + t * CH
                lb = sp.tile([P, CH], F32, tag="lb", name="lb", bufs=2)
                nc.vector.tensor_scalar_max(lb[:], ps_[:], 1e-38)
                nc.scalar.activation(lb[:], lb[:], AF.Ln)
                lq = sp.tile([P, CH], F32, tag="lq", name="lq", bufs=2)
                nc.vector.tensor_scalar_max(lq[:], Q4[:, c0:c0 + CH], 1e-38)
                lnq = sp.tile([P, CH], F32, tag="lnq", name="lnq", bufs=2)
                nc.scalar.activation(lnq[:], lq[:], AF.Ln)
                df = sp.tile([P, CH], F32, tag="lnq", name="df", bufs=2)
                nc.vector.tensor_tensor(df[:], lnq[:], lb[:], OP.subtract)
                nc.vector.tensor_tensor(df[:], df[:], lq[:], OP.mult)
                nc.vector.reduce_sum(kacc[:, m * NCH + t:m * NCH + t + 1],
                                     df[:], axis=AX.X, op=OP.add)

            B2T = transpose_shard(aginB2)
            square(B2T, agoB2, consumer=kld_consumer)

            # ---------------- per-core output (host sums partials) ---------
            ksum = mp.tile([P, 1], F32)
            nc.vector.reduce_sum(ksum[:], kacc[:, :], axis=AX.X, op=OP.add)
            pk = pone(1)
            nc.tensor.matmul(pk[:], ksum[:, :], ones_col[:, 0:1],
                             start=True, stop=True)
            out_sb = mp.tile([1, 2], F32)
            nc.scalar.copy(out_sb[0:1, 0:1], pk[:])
            nc.scalar.copy(out_sb[0:1, 1:2], recon_sc[:, :])
            nc.sync.dma_start(dOut[:, :], out_sb[:])

    nc.compile()
    return nc


@functools.lru_cache(maxsize=4)
def _built(fs_value: float, debug_names: tuple = ()):
    return _build(fs_value, debug_names)


def _in_maps(inputs):
    X = np.ascontiguousarray(inputs["X"], dtype=np.float32)
    base = {"X": X,
            "flows": np.ascontiguousarray(inputs["flows"], dtype=np.float32),
            "ones2048": np.ones((1, N), dtype=np.float32)}
    for nm, sh in WSPECS:
        base[nm] = np.ascontiguousarray(
            np.asarray(inputs[nm], dtype=np.float32).reshape(sh))
    maps = []
    for c in range(NCORES):
        m = dict(base)
        m["Xshard"] = np.ascontiguousarray(X[c * S:(c + 1) * S])
        maps.append(m)
    return maps


def _host_reduce(res):
    kld = 0.0
    for c in range(NCORES):
        kld += float(res.results[c]["out"][0, 0])
    recon = float(res.results[0]["out"][0, 1])
    return np.float32(kld / N + recon)


def kernel(**inputs) -> np.ndarray:
    fs_value = float(np.asarray(inputs["fs"]))
    nc = _built(fs_value)
    maps = _in_maps(inputs)
    res = run_bass_kernel_spmd(nc, maps, core_ids=list(range(NCORES)))
    return np.array(_host_reduce(res), dtype=np.float32)
